# revision 56
# baseline (speedup 1.0000x reference)
"""Multi-head attention forward on 8 Trainium2 NeuronCores (Bass/Tile).

Problem: B=2, N=2048, DIM=1024, 16 heads x 64. Sharding: core i handles
batch b = i//4 and head-group g = i%4 (4 heads = 256 channels).

Per-core dataflow (matmuls in float32r — full PE rate, ~1.5e-4 precision —
with fp32 PSUM accumulation; inputs DMA directly into f32r SBUF tiles):
  QT/KT = Wg @ x^T            [z=256, n=2048]  (z-major so scores contract d)
  V     = x @ Wv_g^T          [n=2048, z=256]  (natural, + ones column / head)
  S^T   = K_h Q_h^T           [k=128-tile, q=512-chunk]; the pair of heads in
                               a z-tile is packed in one PE pass via
                               tile_position row tiling (K=64 each)
  E     = exp(0.125 * S^T)    (ACT, one call per [128, 1024] pair tile)
  O^T|s = [V_h|1]^T E_h       PSUM-accumulated over 16 k-tiles, M=65
  ao    = O^T * (1/s)         (DVE recip + gpsimd partition broadcast)
  pT    = Wp_g @ aoT          [c=1024, n=2048] partial output, host-reduced

Host: out[b] = sum_g pT(b,g)^T + bp.
"""
import os
import sys
from contextlib import ExitStack

import numpy as np

for _p in ("/opt/trn_rl_repo", "/root/.axon_site/_ro/trn_rl_repo"):
    if os.path.isdir(_p) and _p not in sys.path:
        sys.path.insert(0, _p)

SEQ = 2048      # sequence length
C = 1024        # model dim
ZL = 256        # channels per head-group (4 heads x 64)
NCORES = 8
CT = C // 128   # contraction tiles for projections
QC = SEQ // 512  # q chunks
KT = SEQ // 128  # k-seq tiles
NP = 2          # head pairs per core
SCALE = 0.125   # HEAD_DIM ** -0.5

_STATE = {}


def _emit(nc, tc, ctx, aps):
    from concourse import mybir

    f32 = mybir.dt.float32
    f32r = mybir.dt.float32r
    AF = mybir.ActivationFunctionType
    xT, wT, wpT, pT = aps["xT"], aps["w"], aps["wpT"], aps["pT"]

    sb = ctx.enter_context(tc.tile_pool(name="sb", bufs=1))
    ps = ctx.enter_context(tc.tile_pool(name="ps", bufs=1, space="PSUM"))

    ones = sb.tile([128, 1], f32, tag="ones", bufs=1)
    nc.vector.memset(ones, 1.0)

    # PE warmup during the initial DMA wait: dummy f32 matmuls (4 cyc/row,
    # so few instructions cover the ~3us HAM window) keep the activity
    # monitor busy so the first real matmuls run at 2.4 GHz.
    warmf = sb.tile([128, 512], f32, tag="warmf", bufs=1)
    nc.vector.memset(warmf, 1.0)
    wps_ = ps.tile([128, 512], f32, tag="st", bufs=2, name="warmps")
    for _ in range(4):
        nc.tensor.matmul(wps_, warmf[:, 0:128], warmf, start=True, stop=True)

    # ---- DMAs: x chunk 0 first so QKV starts immediately; wp last ----
    xTt = xT.rearrange("(t p) n -> p t n", p=128)
    xr_t = []
    w_r = [None] * 3

    def dma_x(j, split_after_half=None):
        xr = sb.tile([128, CT, 512], f32r, tag="xr", bufs=2, name="xr")
        src = xTt[:, :, j * 512:(j + 1) * 512]
        if split_after_half is not None:  # first c-tiles land sooner
            h = CT // 2
            nc.sync.dma_start(out=xr[:, 0:h, :], in_=src[:, 0:h, :])
            split_after_half()
            nc.sync.dma_start(out=xr[:, h:CT, :], in_=src[:, h:CT, :])
        else:
            nc.sync.dma_start(out=xr, in_=src)
        xr_t.append(xr)

    def dma_w(wi):
        wr = sb.tile([128, CT, ZL], f32r, tag=f"w{wi}", bufs=1, name=f"w{wi}")
        nc.sync.dma_start(out=wr, in_=wT[wi].rearrange("(t p) z -> p t z", p=128))
        w_r[wi] = wr

    dma_x(0, split_after_half=lambda: dma_w(0))
    dma_w(1)
    dma_w(2)
    for j in (1, 2, 3):
        dma_x(j)
    wp_r = sb.tile([128, 2, C], f32r, tag="wp", bufs=1)
    nc.sync.dma_start(out=wp_r, in_=wpT.rearrange("(t p) c -> p t c", p=128))

    # ---- QKV projections for one seq chunk ----
    qt = [[None] * QC for _ in range(NP)]   # [pair][chunk] -> [128, 512] f32r
    kt = [[None] * QC for _ in range(NP)]
    vt = [None] * KT                        # [k-tile] -> [128, 4*65] f32r

    def proj_qk(wi, p, j):
        xr = xr_t[j]
        acc = ps.tile([128, 512], f32, tag="mm", bufs=2, name="acc")
        for ct in range(CT):
            nc.tensor.matmul(
                acc, w_r[wi][:, ct, p * 128:(p + 1) * 128], xr[:, ct, :],
                start=(ct == 0), stop=(ct == CT - 1))
        d = sb.tile([128, 512], f32r, tag="qk", bufs=2 * NP * QC, name="qk")
        nc.vector.tensor_copy(d, acc)
        (qt if wi == 0 else kt)[p][j] = d

    def proj_v(j):
        xr = xr_t[j]
        for nl in range(4):  # V k-seq tiles of this chunk
            acc = ps.tile([128, ZL], f32, tag="mm", bufs=2, name="acc")
            for ct in range(CT):
                nc.tensor.matmul(
                    acc, xr[:, ct, nl * 128:(nl + 1) * 128], w_r[2][:, ct, :],
                    start=(ct == 0), stop=(ct == CT - 1))
            v = sb.tile([128, 4 * 65], f32r, tag="vt", bufs=KT)
            vv = v.rearrange("p (h e) -> p h e", e=65)
            av = acc.rearrange("p (h d) -> p h d", d=64)
            nc.vector.tensor_copy(vv[:, :, 0:64], av)
            nc.vector.tensor_copy(vv[:, :, 64:65], ones[:, None, :].broadcast_to([128, 4, 1]))
            vt[j * 4 + nl] = v

    # ---- attention (pair-packed scores, exp, PV) + projection ----
    ao = [[None] * QC for _ in range(NP)]   # [z-tile][chunk]

    def alloc_pv():
        return [ps.tile([128, 512], f32, tag="pv", bufs=2, name=f"pv{h}")
                for h in range(2)]

    def score_exp(p, j, k):
        st = ps.tile([128, 1024], f32, tag="st", bufs=2)
        lk = kt[p][k // 4][:, (k % 4) * 128:(k % 4 + 1) * 128]
        nc.tensor.matmul(st[:, 0:512], lk[0:64, :], qt[p][j][0:64, :],
                         start=True, stop=True, tile_position=(0, 0))
        nc.tensor.matmul(st[:, 512:1024], lk[64:128, :], qt[p][j][64:128, :],
                         start=True, stop=True, tile_position=(64, 0))
        e = sb.tile([128, 1024], f32r, tag="e", bufs=14)
        nc.scalar.activation(out=e, in_=st, func=AF.Exp, scale=SCALE)
        return e

    def pv_accum(p, pv, k, e):
        for h in range(2):
            hl = 2 * p + h
            nc.tensor.matmul(
                pv[h][0:65, :], vt[k][:, hl * 65:(hl + 1) * 65],
                e[:, h * 512:(h + 1) * 512],
                start=(k == 0), stop=(k == KT - 1))

    def attention_part(p, j, pv, k0, k1):
        for k in range(k0, k1):
            pv_accum(p, pv, k, score_exp(p, j, k))

    def normalize(p, j, pv, last=False):
        # Drain PSUM first with plain copies so the pv banks free up for the
        # next block immediately; the recip/broadcast/mul chain then runs off
        # the critical path from SBUF. For the last block latency matters
        # instead: skip the drain copies and multiply straight from PSUM.
        aot = sb.tile([128, 512], f32r, tag="ao", bufs=4)
        src, s_t, r_t, rb_t = [], [], [], []
        for h in range(2):
            s = sb.tile([1, 512], f32, tag="s", bufs=2)
            nc.vector.tensor_copy(s, pv[h][64:65, :])       # sums, shift 64 -> 0
            s_t.append(s)
            if last:
                src.append(pv[h][0:64, :])
            else:
                cp = sb.tile([64, 512], f32, tag="cp", bufs=2, name="cp")
                nc.vector.tensor_copy(cp, pv[h][0:64, :])   # O^T, releases pv
                src.append(cp)
        for h in range(2):
            r = sb.tile([1, 512], f32, tag="r", bufs=2)
            nc.vector.reciprocal(r, s_t[h])
            rb = sb.tile([64, 512], f32, tag="rb", bufs=2)
            nc.gpsimd.partition_broadcast(rb, r)
            rb_t.append(rb)
        nc.vector.tensor_mul(aot[0:64, :], src[0], rb_t[0])
        tb = sb.tile([64, 512], f32, tag="tb", bufs=2)
        nc.vector.tensor_mul(tb, src[1], rb_t[1])
        nc.vector.tensor_copy(aot[64:128, :], tb)           # shift 0 -> 64
        ao[p][j] = aot

    def proj(j, last=False):
        for m in range(CT):
            acc = ps.tile([128, 512], f32, tag="mm", bufs=2)
            for zt in range(2):
                nc.tensor.matmul(acc, wp_r[:, zt, m * 128:(m + 1) * 128],
                                 ao[zt][j], start=(zt == 0), stop=(zt == 1))
            pstg = sb.tile([128, 512], f32, tag="pstg", bufs=3)
            nc.vector.tensor_copy(pstg, acc)
            nc.sync.dma_start(
                out=pT[m * 128:(m + 1) * 128, j * 512:(j + 1) * 512], in_=pstg)

    # The first attention block is split across the QKV chunks so its score
    # k-tiles are emitted right after the K/V chunk that produces them — the
    # ACT engine starts softmax exps inside the QKV phase. PSUM budget (8
    # banks: st 4 + pv 2 + mm 2) allows exactly one open attention block
    # during QKV; the rest run atomically with proj interleaved.
    # Block (1,0) additionally pre-computes scores+exps for its first 6
    # k-tiles during the later QKV chunks (E tiles buffered in SBUF), so ACT
    # stays fed; its PV accumulation starts once block (0,0) frees the pv
    # banks.
    # K projections are emitted first within each chunk and block (0,0)'s
    # score/exp k-tiles immediately after them, so ACT starts softmax work
    # several microseconds earlier per chunk; Q and V follow (the PVs wait on
    # V through buffered E tiles).
    pv00 = None
    e10 = []
    for j in range(QC):
        if j == 0:
            proj_qk(0, 0, 0)          # qt[0][0] needed by the first scores
        proj_qk(1, 0, j)              # kt[0][j]
        if j == 0:
            pv00 = alloc_pv()
        e00 = [score_exp(0, 0, k) for k in range(4 * j, 4 * (j + 1))]
        proj_qk(1, 1, j)              # kt[1][j]
        if j == 1:
            e10 += [score_exp(1, 0, k) for k in range(0, 4)]
        elif j == 2:
            e10 += [score_exp(1, 0, k) for k in range(4, 8)]
        elif j == 3:
            e10 += [score_exp(1, 0, k) for k in range(8, 12)]
        if j > 0:
            proj_qk(0, 0, j)
        proj_qk(0, 1, j)
        proj_v(j)
        for k, e in zip(range(4 * j, 4 * (j + 1)), e00):
            pv_accum(0, pv00, k, e)
    normalize(0, 0, pv00)

    # Steady state: one attention block at a time (pv bank limit); each
    # chunk's projection is emitted after the *following* block so the PE
    # gap-fills projection matmuls without starving ACT.
    steady = [(1, 0), (0, 1), (1, 1), (0, 2), (1, 2), (0, 3), (1, 3)]
    pre = {b: [] for b in steady}
    pre[(1, 0)] = e10
    pending_proj = None
    for bi, (p, j) in enumerate(steady):
        pv = alloc_pv()
        elist = pre[(p, j)]
        for k, e in enumerate(elist):
            pv_accum(p, pv, k, e)
        nxt = steady[bi + 1] if bi + 1 < len(steady) else None
        for k in range(len(elist), KT):
            pv_accum(p, pv, k, score_exp(p, j, k))
        if nxt is not None:
            pre[nxt] = [score_exp(nxt[0], nxt[1], kk) for kk in range(2)]
        normalize(p, j, pv, last=(bi == len(steady) - 1))
        if pending_proj is not None:
            proj(pending_proj)   # emitted after the following block: PE
            pending_proj = None  # gap-fills it without starving ACT
        if p == 1:
            pending_proj = j
    proj(QC - 1, last=True)


def _build():
    import concourse.tile as tile
    from concourse import bacc, mybir

    f32 = mybir.dt.float32
    f32r = mybir.dt.float32r
    nc = bacc.Bacc("TRN2", target_bir_lowering=False, debug=False,
                   num_devices=NCORES)
    aps = {
        "xT": nc.dram_tensor("xT", [C, SEQ], f32r, kind="ExternalInput").ap(),
        "w": [nc.dram_tensor(n, [C, ZL], f32r, kind="ExternalInput").ap()
              for n in ("wqT", "wkT", "wvT")],
        "wpT": nc.dram_tensor("wpT", [ZL, C], f32r, kind="ExternalInput").ap(),
        "pT": nc.dram_tensor("pT", [C, SEQ], f32, kind="ExternalOutput").ap(),
    }
    with tile.TileContext(nc) as tc, ExitStack() as ctx:
        _emit(nc, tc, ctx, aps)
    nc.compile()
    return nc


def get_nc():
    if "nc" not in _STATE:
        _STATE["nc"] = _build()
    return _STATE["nc"]


def make_in_maps(x, Wq, Wk, Wv, Wp):
    in_maps = []
    for core in range(NCORES):
        b, g = core // 4, core % 4
        sl = slice(g * ZL, (g + 1) * ZL)
        in_maps.append({
            "xT": np.ascontiguousarray(x[b].T),
            "wqT": np.ascontiguousarray(Wq[sl, :].T),
            "wkT": np.ascontiguousarray(Wk[sl, :].T),
            "wvT": np.ascontiguousarray(Wv[sl, :].T),
            "wpT": np.ascontiguousarray(Wp[:, sl].T),
        })
    return in_maps


def kernel(x, Wq, Wk, Wv, Wp, bp):
    from concourse.bass_utils import run_bass_kernel_spmd

    x = np.asarray(x, np.float32)
    Wq, Wk, Wv, Wp, bp = (np.asarray(a, np.float32) for a in (Wq, Wk, Wv, Wp, bp))
    nc = get_nc()
    res = run_bass_kernel_spmd(nc, make_in_maps(x, Wq, Wk, Wv, Wp),
                               core_ids=list(range(NCORES)))
    out = np.zeros((2, SEQ, C), np.float32)
    for core in range(NCORES):
        out[core // 4] += res.results[core]["pT"].T
    out += bp
    return out
